# revision 3
# baseline (speedup 1.0000x reference)
"""AdderVDSR kernel v2 for 8 TRN2 NeuronCores.

Mathematical collapse (see baseline): every AdderNet block outputs exactly 0
in fp32, so reference == pixel_shuffle(conv3(x, up_w, up_b), 2) + out_b.

v2 layout: block-diagonal band stacking.  Core i handles pre-shuffle rows
[16i, 16i+16) of both batches = 4096 pixels, split into 8 bands of 4 rows x
128 cols (band j = (batch, quad)).  Band j owns SBUF partitions [10j, 10j+10):
rows (ci, kw) = host-side im2col over input-channel and kw only (9 rows) plus
a ones row (bias).  The kh taps are free-dim COLUMN SHIFTS of one stored
[80, 768] patch tensor (6 rows x 128 per band incl. halo), so the whole conv
is 3 accumulating matmuls [80, 96] x [80, 512] into one PSUM bank -- 1536 PE
column-cycles total (baseline: 8192).  M order (dr, dc, band, color) makes
the 4 pixel-shuffle interleave copies contiguous-partition [24, 512] slices,
and the output SBUF/DRAM layout [24 = (b, quad, color), 2048] gives 24
contiguous 8KB DMA descriptors split across the two HWDGE queues (SP + ACT).
Host reshapes the flat [24, 2048] per-core result to [2, 3, 32, 256].
"""

import numpy as np

import concourse.bass as bass
import concourse.mybir as mybir
from concourse.bass_utils import run_bass_kernel_spmd

N_CORES = 8
B, C, H, W = 2, 3, 128, 128
NB = 8                       # bands per core: (batch, quad)
KPB = 10                     # partitions per band: 3 ci x 3 kw + ones
K = NB * KPB                 # 80 contraction partitions
M = 128                      # 4 quadrants of 32: (dr, dc) -> 24 used + 8 pad
PCOLS = 6 * W                # 768 patch cols per partition (6 rows x 128)
WCOLS = 3 * M                # 384 weight cols (3 kh blocks of 128)
XCOLS = PCOLS + WCOLS        # 1152

_f32 = mybir.dt.float32
_bf16 = mybir.dt.bfloat16


def build_graph():
    nc = bass.Bass()
    xk = nc.declare_dram_parameter("xk", [K, XCOLS], _bf16, isOutput=False)
    out = nc.declare_dram_parameter("out", [24, 2048], _f32, isOutput=True)

    with (
        nc.sbuf_tensor([K, XCOLS], _bf16) as X,
        nc.sbuf_tensor([24, 2048], _f32) as S,
        nc.sbuf_tensor([1, 16], _f32) as scratch_a,
        nc.sbuf_tensor([1, 16], _f32) as scratch_b,
        nc.psum_tensor([M, 512], _f32) as PS,
        nc.semaphore("in1") as in1,
        nc.semaphore("in2") as in2,
        nc.semaphore("mm_sem") as mm_sem,
        nc.semaphore("cpv") as cpv,
        nc.semaphore("cps") as cps,
        nc.semaphore("outs") as outs,
        nc.Block() as block,
    ):
        S3 = S.rearrange("p (rr x) -> p rr x", rr=4, x=512)
        PS3 = PS.rearrange("p (rr w) -> p rr w", rr=4, w=128)

        def dst_view(dr, dc):
            # col = rr*512 + dr*256 + dc*128 + w (host interleaves w/dc later)
            return S3[0:24, :, 256 * dr + 128 * dc : 256 * dr + 128 * dc + 128]

        def src_view(dr, dc):
            q = 32 * (2 * dr + dc)
            return PS3[q : q + 24, :, :]

        @block.sync
        def _(sync):
            sync.dma_start(out=X[0:44, :], in_=xk[0:44, :]).then_inc(in1, 16)
            sync.wait_ge(cpv, 2)
            sync.wait_ge(cps, 2)
            sync.dma_start(out=out[0:12, :], in_=S[0:12, :]).then_inc(outs, 16)

        @block.scalar
        def _(scalar):
            scalar.dma_start(out=X[44:80, :], in_=xk[44:80, :]).then_inc(in2, 16)
            # Dummy copy pulls ACT_TABLE_LOAD off the post-matmul critical path.
            scalar.copy(scratch_a[0:1, :], scratch_b[0:1, :])
            scalar.wait_ge(mm_sem, 1)
            scalar.copy(dst_view(1, 0), src_view(1, 0)).then_inc(cps, 1)
            scalar.copy(dst_view(1, 1), src_view(1, 1)).then_inc(cps, 1)
            scalar.wait_ge(cps, 2)
            scalar.wait_ge(cpv, 2)
            scalar.dma_start(out=out[12:24, :], in_=S[12:24, :]).then_inc(outs, 16)

        @block.vector
        def _(vector):
            vector.wait_ge(mm_sem, 1)
            vector.tensor_copy(dst_view(0, 0), src_view(0, 0)).then_inc(cpv, 1)
            vector.tensor_copy(dst_view(0, 1), src_view(0, 1)).then_inc(cpv, 1)

        @block.tensor
        def _(tensor):
            tensor.wait_ge(in1, 16)
            tensor.wait_ge(in2, 16)
            for kh in range(3):
                mm = tensor.matmul(
                    PS[0:M, 0:512],
                    lhsT=X[:, PCOLS + M * kh : PCOLS + M * (kh + 1)],
                    rhs=X[:, W * kh : W * kh + 512],
                    start=(kh == 0),
                    stop=(kh == 2),
                )
            mm.then_inc(mm_sem, 1)

    return nc


def make_in_maps(x, up_w, up_b, out_b):
    """Per-core [K, XCOLS] bf16: kw-im2col patch bands + block-diag weights."""
    import ml_dtypes

    x = np.asarray(x, dtype=np.float32)
    up_w = np.asarray(up_w, dtype=np.float32)
    up_b = np.asarray(up_b, dtype=np.float32)
    out_b = np.asarray(out_b, dtype=np.float32)

    # weights: wk[kh][10j + 3ci + kw, 48dr + 24dc + 3j + co]
    wk = np.zeros((3, K, M), dtype=np.float32)
    for j in range(NB):
        for co in range(C):
            for dr in range(2):
                for dc in range(2):
                    o = co * 4 + dr * 2 + dc
                    col = 32 * (2 * dr + dc) + 3 * j + co
                    for ci in range(C):
                        for kw in range(3):
                            wk[:, 10 * j + 3 * ci + kw, col] = up_w[o, ci, :, kw]
                    wk[1, 10 * j + 9, col] = up_b[o] + out_b[co]
    wflat = wk.transpose(1, 0, 2).reshape(K, WCOLS)  # cols (kh, m)

    xpad = np.zeros((B, C, H + 2, W + 2), dtype=np.float32)
    xpad[:, :, 1 : H + 1, 1 : W + 1] = x

    in_maps = []
    for i in range(N_CORES):
        xc = np.empty((K, XCOLS), dtype=np.float32)
        xc[:, PCOLS:] = wflat
        for j in range(NB):
            b, quad = divmod(j, 4)
            r0 = 16 * i + 4 * quad
            for ci in range(C):
                for kw in range(3):
                    xc[10 * j + 3 * ci + kw, :PCOLS] = xpad[
                        b, ci, r0 : r0 + 6, kw : kw + W
                    ].reshape(PCOLS)
            xc[10 * j + 9, :PCOLS] = 1.0
        in_maps.append({"xk": xc.astype(ml_dtypes.bfloat16)})
    return in_maps


def kernel(x, up_w, up_b, in_w, in_b, adder_w, out_w, out_b):
    nc = build_graph()
    in_maps = make_in_maps(x, up_w, up_b, out_b)
    res = run_bass_kernel_spmd(nc, in_maps, core_ids=list(range(N_CORES)))
    slabs = []
    for i in range(N_CORES):
        a = np.asarray(res.results[i]["out"])  # [24, (rr dr dc w)]
        a = a.reshape(2, 4, 3, 4, 2, 2, 128)   # b quad co rr dr dc w
        a = a.transpose(0, 2, 1, 3, 4, 6, 5)   # b co quad rr dr w dc
        a = a.reshape(2, 3, 32, 256)
        slabs.append(a)
    return np.concatenate(slabs, axis=2).astype(np.float32)


# revision 4
# speedup vs baseline: 1.0709x; 1.0709x over previous
"""AdderVDSR kernel v2 for 8 TRN2 NeuronCores.

Mathematical collapse (see baseline): every AdderNet block outputs exactly 0
in fp32, so reference == pixel_shuffle(conv3(x, up_w, up_b), 2) + out_b.

v2 layout: block-diagonal band stacking.  Core i handles pre-shuffle rows
[16i, 16i+16) of both batches = 4096 pixels, split into 8 bands of 4 rows x
128 cols (band j = (batch, quad)).  Band j owns SBUF partitions [10j, 10j+10):
rows (ci, kw) = host-side im2col over input-channel and kw only (9 rows) plus
a ones row (bias).  The kh taps are free-dim COLUMN SHIFTS of one stored
[80, 768] patch tensor (6 rows x 128 per band incl. halo), so the whole conv
is 3 accumulating matmuls [80, 96] x [80, 512] into one PSUM bank -- 1536 PE
column-cycles total (baseline: 8192).  M order (dr, dc, band, color) makes
the 4 pixel-shuffle interleave copies contiguous-partition [24, 512] slices,
and the output SBUF/DRAM layout [24 = (b, quad, color), 2048] gives 24
contiguous 8KB DMA descriptors split across the two HWDGE queues (SP + ACT).
Host reshapes the flat [24, 2048] per-core result to [2, 3, 32, 256].
"""

import os

os.environ["CONCOURSE_SCRUB_NEFF_DEBUG_INFO"] = "1"

import numpy as np

import concourse.bass as bass
import concourse.mybir as mybir
from concourse.bass_utils import run_bass_kernel_spmd

N_CORES = 8
B, C, H, W = 2, 3, 128, 128
NB = 8                       # bands per core: (batch, quad)
KPB = 10                     # partitions per band: 3 ci x 3 kw + ones
K = NB * KPB                 # 80 contraction partitions
M = 128                      # 4 quadrants of 32: (dr, dc) -> 24 used + 8 pad
PCOLS = 6 * W                # 768 patch cols per partition (6 rows x 128)
WCOLS = 3 * M                # 384 weight cols (3 kh blocks of 128)
XCOLS = PCOLS + WCOLS        # 1152

_f32 = mybir.dt.float32
_bf16 = mybir.dt.bfloat16


def build_graph():
    nc = bass.Bass(disable_frame_to_traceback=True)
    xk = nc.declare_dram_parameter("xk", [K, XCOLS], _bf16, isOutput=False)
    out = nc.declare_dram_parameter("out", [24, 2048], _f32, isOutput=True)

    with (
        nc.sbuf_tensor([K, XCOLS], _bf16) as X,
        nc.sbuf_tensor([24, 2048], _f32) as S,
        nc.sbuf_tensor([1, 16], _f32) as scratch_a,
        nc.sbuf_tensor([1, 16], _f32) as scratch_b,
        nc.psum_tensor([M, 512], _f32) as PS,
        nc.semaphore("in1") as in1,
        nc.semaphore("in2") as in2,
        nc.semaphore("mm_sem") as mm_sem,
        nc.semaphore("cpv") as cpv,
        nc.semaphore("cps") as cps,
        nc.semaphore("outs") as outs,
        nc.Block() as block,
    ):
        S3 = S.rearrange("p (rr x) -> p rr x", rr=4, x=512)
        PS3 = PS.rearrange("p (rr w) -> p rr w", rr=4, w=128)

        def dst_view(dr, dc):
            # col = rr*512 + dr*256 + dc*128 + w (host interleaves w/dc later)
            return S3[0:24, :, 256 * dr + 128 * dc : 256 * dr + 128 * dc + 128]

        def src_view(dr, dc):
            q = 32 * (2 * dr + dc)
            return PS3[q : q + 24, :, :]

        @block.sync
        def _(sync):
            sync.dma_start(out=X[0:44, :], in_=xk[0:44, :]).then_inc(in1, 16)
            sync.wait_ge(cpv, 2)
            sync.wait_ge(cps, 2)
            sync.dma_start(out=out[0:12, :], in_=S[0:12, :]).then_inc(outs, 16)

        @block.scalar
        def _(scalar):
            scalar.dma_start(out=X[44:80, :], in_=xk[44:80, :]).then_inc(in2, 16)
            # Dummy copy pulls ACT_TABLE_LOAD off the post-matmul critical path.
            scalar.copy(scratch_a[0:1, :], scratch_b[0:1, :])
            scalar.wait_ge(mm_sem, 1)
            scalar.copy(dst_view(1, 0), src_view(1, 0)).then_inc(cps, 1)
            scalar.copy(dst_view(1, 1), src_view(1, 1)).then_inc(cps, 1)
            scalar.wait_ge(cps, 2)
            scalar.wait_ge(cpv, 2)
            scalar.dma_start(out=out[12:24, :], in_=S[12:24, :]).then_inc(outs, 16)

        @block.vector
        def _(vector):
            vector.wait_ge(mm_sem, 1)
            vector.tensor_copy(dst_view(0, 0), src_view(0, 0)).then_inc(cpv, 1)
            vector.tensor_copy(dst_view(0, 1), src_view(0, 1)).then_inc(cpv, 1)

        @block.tensor
        def _(tensor):
            tensor.wait_ge(in1, 16)
            tensor.wait_ge(in2, 16)
            for kh in range(3):
                mm = tensor.matmul(
                    PS[0:M, 0:512],
                    lhsT=X[:, PCOLS + M * kh : PCOLS + M * (kh + 1)],
                    rhs=X[:, W * kh : W * kh + 512],
                    start=(kh == 0),
                    stop=(kh == 2),
                )
            mm.then_inc(mm_sem, 1)

    return nc


def make_in_maps(x, up_w, up_b, out_b):
    """Per-core [K, XCOLS] bf16: kw-im2col patch bands + block-diag weights."""
    import ml_dtypes

    x = np.asarray(x, dtype=np.float32)
    up_w = np.asarray(up_w, dtype=np.float32)
    up_b = np.asarray(up_b, dtype=np.float32)
    out_b = np.asarray(out_b, dtype=np.float32)

    # weights: wk[kh][10j + 3ci + kw, 48dr + 24dc + 3j + co]
    wk = np.zeros((3, K, M), dtype=np.float32)
    for j in range(NB):
        for co in range(C):
            for dr in range(2):
                for dc in range(2):
                    o = co * 4 + dr * 2 + dc
                    col = 32 * (2 * dr + dc) + 3 * j + co
                    for ci in range(C):
                        for kw in range(3):
                            wk[:, 10 * j + 3 * ci + kw, col] = up_w[o, ci, :, kw]
                    wk[1, 10 * j + 9, col] = up_b[o] + out_b[co]
    wflat = wk.transpose(1, 0, 2).reshape(K, WCOLS)  # cols (kh, m)

    xpad = np.zeros((B, C, H + 2, W + 2), dtype=np.float32)
    xpad[:, :, 1 : H + 1, 1 : W + 1] = x

    in_maps = []
    for i in range(N_CORES):
        xc = np.empty((K, XCOLS), dtype=np.float32)
        xc[:, PCOLS:] = wflat
        for j in range(NB):
            b, quad = divmod(j, 4)
            r0 = 16 * i + 4 * quad
            for ci in range(C):
                for kw in range(3):
                    xc[10 * j + 3 * ci + kw, :PCOLS] = xpad[
                        b, ci, r0 : r0 + 6, kw : kw + W
                    ].reshape(PCOLS)
            xc[10 * j + 9, :PCOLS] = 1.0
        in_maps.append({"xk": xc.astype(ml_dtypes.bfloat16)})
    return in_maps


def kernel(x, up_w, up_b, in_w, in_b, adder_w, out_w, out_b):
    nc = build_graph()
    in_maps = make_in_maps(x, up_w, up_b, out_b)
    res = run_bass_kernel_spmd(nc, in_maps, core_ids=list(range(N_CORES)))
    slabs = []
    for i in range(N_CORES):
        a = np.asarray(res.results[i]["out"])  # [24, (rr dr dc w)]
        a = a.reshape(2, 4, 3, 4, 2, 2, 128)   # b quad co rr dr dc w
        a = a.transpose(0, 2, 1, 3, 4, 6, 5)   # b co quad rr dr w dc
        a = a.reshape(2, 3, 32, 256)
        slabs.append(a)
    return np.concatenate(slabs, axis=2).astype(np.float32)


# revision 5
# speedup vs baseline: 1.0892x; 1.0171x over previous
"""AdderVDSR kernel v2 for 8 TRN2 NeuronCores.

Mathematical collapse (see baseline): every AdderNet block outputs exactly 0
in fp32, so reference == pixel_shuffle(conv3(x, up_w, up_b), 2) + out_b.

v2 layout: block-diagonal band stacking.  Core i handles pre-shuffle rows
[16i, 16i+16) of both batches = 4096 pixels, split into 8 bands of 4 rows x
128 cols (band j = (batch, quad)).  Band j owns SBUF partitions [10j, 10j+10):
rows (ci, kw) = host-side im2col over input-channel and kw only (9 rows) plus
a ones row (bias).  The kh taps are free-dim COLUMN SHIFTS of one stored
[80, 768] patch tensor (6 rows x 128 per band incl. halo), so the whole conv
is 3 accumulating matmuls [80, 96] x [80, 512] into one PSUM bank -- 1536 PE
column-cycles total (baseline: 8192).  M order (dr, dc, band, color) makes
the 4 pixel-shuffle interleave copies contiguous-partition [24, 512] slices,
and the output SBUF/DRAM layout [24 = (b, quad, color), 2048] gives 24
contiguous 8KB DMA descriptors split across the two HWDGE queues (SP + ACT).
Host reshapes the flat [24, 2048] per-core result to [2, 3, 32, 256].
"""

import os

os.environ["CONCOURSE_SCRUB_NEFF_DEBUG_INFO"] = "1"

import numpy as np

import concourse.bass as bass
import concourse.mybir as mybir
from concourse.bass_utils import run_bass_kernel_spmd

N_CORES = 8
B, C, H, W = 2, 3, 128, 128
NB = 8                       # bands per core: (batch, quad)
KPB = 10                     # partitions per band: 3 ci x 3 kw + ones
K = NB * KPB                 # 80 contraction partitions
M = 128                      # 4 quadrants of 32: (dr, dc) -> 24 used + 8 pad
PCOLS = 6 * W                # 768 patch cols per partition (6 rows x 128)
WCOLS = 3 * M                # 384 weight cols (3 kh blocks of 128)
XCOLS = PCOLS + WCOLS        # 1152

_f32 = mybir.dt.float32
_bf16 = mybir.dt.bfloat16


def build_graph():
    nc = bass.Bass(disable_frame_to_traceback=True)
    xk = nc.declare_dram_parameter("xk", [K, XCOLS], _bf16, isOutput=False)
    out = nc.declare_dram_parameter("out", [48, 1024], _f32, isOutput=True)

    with (
        nc.sbuf_tensor([K, XCOLS], _bf16) as X,
        nc.sbuf_tensor([48, 1024], _f32) as S,
        nc.sbuf_tensor([1, 16], _f32) as scratch_a,
        nc.sbuf_tensor([1, 16], _f32) as scratch_b,
        nc.psum_tensor([M, 512], _f32) as PS,
        nc.semaphore("in1") as in1,
        nc.semaphore("in2") as in2,
        nc.semaphore("mm_sem") as mm_sem,
        nc.semaphore("cpv") as cpv,
        nc.semaphore("cps") as cps,
        nc.semaphore("outs") as outs,
        nc.Block() as block,
    ):
        S3 = S.rearrange("p (rr x) -> p rr x", rr=4, x=256)
        PS3 = PS.rearrange("p (rr w) -> p rr w", rr=4, w=128)

        def dst_view(dr):
            # col = rr*256 + dr*128 + w (host interleaves w/dc and splits dc later)
            return S3[0:48, :, 128 * dr : 128 * dr + 128]

        def src_view(dr):
            return PS3[64 * dr : 64 * dr + 48, :, :]

        @block.sync
        def _(sync):
            sync.dma_start(out=X[0:44, :], in_=xk[0:44, :]).then_inc(in1, 16)
            sync.wait_ge(cpv, 1)
            sync.wait_ge(cps, 1)
            sync.dma_start(out=out[0:24, :], in_=S[0:24, :]).then_inc(outs, 16)

        @block.scalar
        def _(scalar):
            scalar.dma_start(out=X[44:80, :], in_=xk[44:80, :]).then_inc(in2, 16)
            # Dummy copy pulls ACT_TABLE_LOAD off the post-matmul critical path.
            scalar.copy(scratch_a[0:1, :], scratch_b[0:1, :])
            scalar.wait_ge(mm_sem, 1)
            scalar.copy(dst_view(1), src_view(1)).then_inc(cps, 1)
            scalar.wait_ge(cps, 1)
            scalar.wait_ge(cpv, 1)
            scalar.dma_start(out=out[24:48, :], in_=S[24:48, :]).then_inc(outs, 16)

        @block.vector
        def _(vector):
            vector.wait_ge(mm_sem, 1)
            vector.tensor_copy(dst_view(0), src_view(0)).then_inc(cpv, 1)

        @block.tensor
        def _(tensor):
            tensor.wait_ge(in1, 16)
            tensor.wait_ge(in2, 16)
            for kh in range(3):
                mm = tensor.matmul(
                    PS[0:M, 0:512],
                    lhsT=X[:, PCOLS + M * kh : PCOLS + M * (kh + 1)],
                    rhs=X[:, W * kh : W * kh + 512],
                    start=(kh == 0),
                    stop=(kh == 2),
                )
            mm.then_inc(mm_sem, 1)

    return nc


def make_in_maps(x, up_w, up_b, out_b):
    """Per-core [K, XCOLS] bf16: kw-im2col patch bands + block-diag weights."""
    import ml_dtypes

    x = np.asarray(x, dtype=np.float32)
    up_w = np.asarray(up_w, dtype=np.float32)
    up_b = np.asarray(up_b, dtype=np.float32)
    out_b = np.asarray(out_b, dtype=np.float32)

    # weights: wk[kh][10j + 3ci + kw, 48dr + 24dc + 3j + co]
    wk = np.zeros((3, K, M), dtype=np.float32)
    for j in range(NB):
        for co in range(C):
            for dr in range(2):
                for dc in range(2):
                    o = co * 4 + dr * 2 + dc
                    col = 64 * dr + 24 * dc + 3 * j + co
                    for ci in range(C):
                        for kw in range(3):
                            wk[:, 10 * j + 3 * ci + kw, col] = up_w[o, ci, :, kw]
                    wk[1, 10 * j + 9, col] = up_b[o] + out_b[co]
    wflat = wk.transpose(1, 0, 2).reshape(K, WCOLS)  # cols (kh, m)

    xpad = np.zeros((B, C, H + 2, W + 2), dtype=np.float32)
    xpad[:, :, 1 : H + 1, 1 : W + 1] = x

    in_maps = []
    for i in range(N_CORES):
        xc = np.empty((K, XCOLS), dtype=np.float32)
        xc[:, PCOLS:] = wflat
        for j in range(NB):
            b, quad = divmod(j, 4)
            r0 = 16 * i + 4 * quad
            for ci in range(C):
                for kw in range(3):
                    xc[10 * j + 3 * ci + kw, :PCOLS] = xpad[
                        b, ci, r0 : r0 + 6, kw : kw + W
                    ].reshape(PCOLS)
            xc[10 * j + 9, :PCOLS] = 1.0
        in_maps.append({"xk": xc.astype(ml_dtypes.bfloat16)})
    return in_maps


def kernel(x, up_w, up_b, in_w, in_b, adder_w, out_w, out_b):
    nc = build_graph()
    in_maps = make_in_maps(x, up_w, up_b, out_b)
    res = run_bass_kernel_spmd(nc, in_maps, core_ids=list(range(N_CORES)))
    slabs = []
    for i in range(N_CORES):
        a = np.asarray(res.results[i]["out"])  # [48 = (dc b quad co), (rr dr w)]
        a = a.reshape(2, 2, 4, 3, 4, 2, 128)   # dc b quad co rr dr w
        a = a.transpose(1, 3, 2, 4, 5, 6, 0)   # b co quad rr dr w dc
        a = a.reshape(2, 3, 32, 256)
        slabs.append(a)
    return np.concatenate(slabs, axis=2).astype(np.float32)


# revision 6
# speedup vs baseline: 1.1210x; 1.0293x over previous
"""AdderVDSR kernel v2 for 8 TRN2 NeuronCores.

Mathematical collapse (see baseline): every AdderNet block outputs exactly 0
in fp32, so reference == pixel_shuffle(conv3(x, up_w, up_b), 2) + out_b.

v2 layout: block-diagonal band stacking.  Core i handles pre-shuffle rows
[16i, 16i+16) of both batches = 4096 pixels, split into 8 bands of 4 rows x
128 cols (band j = (batch, quad)).  Band j owns SBUF partitions [10j, 10j+10):
rows (ci, kw) = host-side im2col over input-channel and kw only (9 rows) plus
a ones row (bias).  The kh taps are free-dim COLUMN SHIFTS of one stored
[80, 768] patch tensor (6 rows x 128 per band incl. halo), so the whole conv
is 3 accumulating matmuls [80, 96] x [80, 512] into one PSUM bank -- 1536 PE
column-cycles total (baseline: 8192).  M order (dr, dc, band, color) makes
the 4 pixel-shuffle interleave copies contiguous-partition [24, 512] slices,
and the output SBUF/DRAM layout [24 = (b, quad, color), 2048] gives 24
contiguous 8KB DMA descriptors split across the two HWDGE queues (SP + ACT).
Host reshapes the flat [24, 2048] per-core result to [2, 3, 32, 256].
"""

import os

os.environ["CONCOURSE_SCRUB_NEFF_DEBUG_INFO"] = "1"

import numpy as np

import concourse.bass as bass
import concourse.mybir as mybir
from concourse.bass_utils import run_bass_kernel_spmd

N_CORES = 8
B, C, H, W = 2, 3, 128, 128
NB = 8                       # bands per core: (batch, quad)
KPB = 10                     # partitions per band: 3 ci x 3 kw + ones
K = NB * KPB                 # 80 contraction partitions
M = 128                      # 4 quadrants of 32: (dr, dc) -> 24 used + 8 pad
PCOLS = 6 * W                # 768 patch cols per partition (6 rows x 128)
WCOLS = 3 * M                # 384 weight cols (3 kh blocks of 128)
XCOLS = PCOLS + WCOLS        # 1152

_f32 = mybir.dt.float32
_bf16 = mybir.dt.bfloat16


def build_graph():
    nc = bass.Bass(disable_frame_to_traceback=True)
    xk = nc.declare_dram_parameter("xk", [K, XCOLS], _bf16, isOutput=False)
    out = nc.declare_dram_parameter("out", [48, 1024], _f32, isOutput=True)

    with (
        nc.sbuf_tensor([K, XCOLS], _bf16) as X,
        nc.sbuf_tensor([48, 1024], _f32) as S,
        nc.sbuf_tensor([1, 16], _f32) as scratch_a,
        nc.sbuf_tensor([1, 16], _f32) as scratch_b,
        nc.psum_tensor([M, 512], _f32) as PS,
        nc.semaphore("in1") as in1,
        nc.semaphore("in2") as in2,
        nc.semaphore("mm_sem") as mm_sem,
        nc.semaphore("cpv") as cpv,
        nc.semaphore("cps") as cps,
        nc.semaphore("outs") as outs,
        nc.Block() as block,
    ):
        S3 = S.rearrange("p (rr x) -> p rr x", rr=4, x=256)
        PS3 = PS.rearrange("p (rr w) -> p rr w", rr=4, w=128)

        def dst_view(dr):
            # col = rr*256 + dr*128 + w (host interleaves w/dc and splits dc later)
            return S3[0:48, :, 128 * dr : 128 * dr + 128]

        def src_view(dr):
            return PS3[64 * dr : 64 * dr + 48, :, :]

        @block.sync
        def _(sync):
            sync.dma_start(out=X[0:44, :], in_=xk[0:44, :]).then_inc(in1, 16)
            sync.wait_ge(cpv, 1)
            sync.wait_ge(cps, 1)
            sync.dma_start(out=out[0:32, :], in_=S[0:32, :]).then_inc(outs, 16)

        @block.scalar
        def _(scalar):
            scalar.dma_start(out=X[44:80, :], in_=xk[44:80, :]).then_inc(in2, 16)
            # Dummy copy pulls ACT_TABLE_LOAD off the post-matmul critical path.
            scalar.copy(scratch_a[0:1, :], scratch_b[0:1, :])
            scalar.wait_ge(mm_sem, 1)
            scalar.copy(dst_view(1), src_view(1)).then_inc(cps, 1)
            scalar.wait_ge(cps, 1)
            scalar.wait_ge(cpv, 1)
            scalar.dma_start(out=out[32:48, :], in_=S[32:48, :]).then_inc(outs, 16)

        @block.vector
        def _(vector):
            vector.wait_ge(mm_sem, 1)
            vector.tensor_copy(dst_view(0), src_view(0)).then_inc(cpv, 1)

        @block.tensor
        def _(tensor):
            tensor.wait_ge(in1, 16)
            tensor.wait_ge(in2, 16)
            for kh in range(3):
                mm = tensor.matmul(
                    PS[0:M, 0:512],
                    lhsT=X[:, PCOLS + M * kh : PCOLS + M * (kh + 1)],
                    rhs=X[:, W * kh : W * kh + 512],
                    start=(kh == 0),
                    stop=(kh == 2),
                )
            mm.then_inc(mm_sem, 1)

    return nc


def make_in_maps(x, up_w, up_b, out_b):
    """Per-core [K, XCOLS] bf16: kw-im2col patch bands + block-diag weights."""
    import ml_dtypes

    x = np.asarray(x, dtype=np.float32)
    up_w = np.asarray(up_w, dtype=np.float32)
    up_b = np.asarray(up_b, dtype=np.float32)
    out_b = np.asarray(out_b, dtype=np.float32)

    # weights: wk[kh][10j + 3ci + kw, 48dr + 24dc + 3j + co]
    wk = np.zeros((3, K, M), dtype=np.float32)
    for j in range(NB):
        for co in range(C):
            for dr in range(2):
                for dc in range(2):
                    o = co * 4 + dr * 2 + dc
                    col = 64 * dr + 24 * dc + 3 * j + co
                    for ci in range(C):
                        for kw in range(3):
                            wk[:, 10 * j + 3 * ci + kw, col] = up_w[o, ci, :, kw]
                    wk[1, 10 * j + 9, col] = up_b[o] + out_b[co]
    wflat = wk.transpose(1, 0, 2).reshape(K, WCOLS)  # cols (kh, m)

    xpad = np.zeros((B, C, H + 2, W + 2), dtype=np.float32)
    xpad[:, :, 1 : H + 1, 1 : W + 1] = x

    in_maps = []
    for i in range(N_CORES):
        xc = np.empty((K, XCOLS), dtype=np.float32)
        xc[:, PCOLS:] = wflat
        for j in range(NB):
            b, quad = divmod(j, 4)
            r0 = 16 * i + 4 * quad
            for ci in range(C):
                for kw in range(3):
                    xc[10 * j + 3 * ci + kw, :PCOLS] = xpad[
                        b, ci, r0 : r0 + 6, kw : kw + W
                    ].reshape(PCOLS)
            xc[10 * j + 9, :PCOLS] = 1.0
        in_maps.append({"xk": xc.astype(ml_dtypes.bfloat16)})
    return in_maps


def kernel(x, up_w, up_b, in_w, in_b, adder_w, out_w, out_b):
    nc = build_graph()
    in_maps = make_in_maps(x, up_w, up_b, out_b)
    res = run_bass_kernel_spmd(nc, in_maps, core_ids=list(range(N_CORES)))
    slabs = []
    for i in range(N_CORES):
        a = np.asarray(res.results[i]["out"])  # [48 = (dc b quad co), (rr dr w)]
        a = a.reshape(2, 2, 4, 3, 4, 2, 128)   # dc b quad co rr dr w
        a = a.transpose(1, 3, 2, 4, 5, 6, 0)   # b co quad rr dr w dc
        a = a.reshape(2, 3, 32, 256)
        slabs.append(a)
    return np.concatenate(slabs, axis=2).astype(np.float32)


# revision 7
# speedup vs baseline: 1.1256x; 1.0040x over previous
"""AdderVDSR kernel v2 for 8 TRN2 NeuronCores.

Mathematical collapse (see baseline): every AdderNet block outputs exactly 0
in fp32, so reference == pixel_shuffle(conv3(x, up_w, up_b), 2) + out_b.

v2 layout: block-diagonal band stacking.  Core i handles pre-shuffle rows
[16i, 16i+16) of both batches = 4096 pixels, split into 8 bands of 4 rows x
128 cols (band j = (batch, quad)).  Band j owns SBUF partitions [10j, 10j+10):
rows (ci, kw) = host-side im2col over input-channel and kw only (9 rows) plus
a ones row (bias).  The kh taps are free-dim COLUMN SHIFTS of one stored
[80, 768] patch tensor (6 rows x 128 per band incl. halo), so the whole conv
is 3 accumulating matmuls [80, 96] x [80, 512] into one PSUM bank -- 1536 PE
column-cycles total (baseline: 8192).  M order (dr, dc, band, color) makes
the 4 pixel-shuffle interleave copies contiguous-partition [24, 512] slices,
and the output SBUF/DRAM layout [24 = (b, quad, color), 2048] gives 24
contiguous 8KB DMA descriptors split across the two HWDGE queues (SP + ACT).
Host reshapes the flat [24, 2048] per-core result to [2, 3, 32, 256].
"""

import os

os.environ["CONCOURSE_SCRUB_NEFF_DEBUG_INFO"] = "1"

import numpy as np

import concourse.bass as bass
import concourse.mybir as mybir
from concourse.bass_utils import run_bass_kernel_spmd

N_CORES = 8
B, C, H, W = 2, 3, 128, 128
NB = 8                       # bands per core: (batch, row-half, col-half)
KPB = 10                     # partitions per band: 3 ci x 3 kw + ones
K = NB * KPB                 # 80 contraction partitions
M = 128                      # two 64-aligned dr groups of 48 used cols
BW = 64                      # band width (cols); band = 8 rows x 64 cols
PCOLS = 10 * BW              # 640 patch cols per partition (10 rows x 64)
WCOLS = 3 * M                # 384 weight cols (3 kh blocks of 128)
XCOLS = PCOLS + WCOLS        # 1024

_f32 = mybir.dt.float32
_bf16 = mybir.dt.bfloat16


def build_graph():
    nc = bass.Bass(disable_frame_to_traceback=True)
    xk = nc.declare_dram_parameter("xk", [K, XCOLS], _bf16, isOutput=False)
    out = nc.declare_dram_parameter("out", [48, 1024], _f32, isOutput=True)

    with (
        nc.sbuf_tensor([K, XCOLS], _bf16) as X,
        nc.sbuf_tensor([48, 1024], _f32) as S,
        nc.sbuf_tensor([1, 16], _f32) as scratch_a,
        nc.sbuf_tensor([1, 16], _f32) as scratch_b,
        nc.psum_tensor([M, 512], _f32) as PS,
        nc.semaphore("in1") as in1,
        nc.semaphore("in2") as in2,
        nc.semaphore("mm_sem") as mm_sem,
        nc.semaphore("cpv") as cpv,
        nc.semaphore("cps") as cps,
        nc.semaphore("outs") as outs,
        nc.Block() as block,
    ):
        S3 = S.rearrange("p (rr x) -> p rr x", rr=4, x=256)
        PS3 = PS.rearrange("p (rr w) -> p rr w", rr=4, w=128)

        def dst_view(dr):
            # col = rr*256 + dr*128 + w (host interleaves w/dc and splits dc later)
            return S3[0:48, :, 128 * dr : 128 * dr + 128]

        def src_view(dr):
            return PS3[64 * dr : 64 * dr + 48, :, :]

        @block.sync
        def _(sync):
            sync.dma_start(out=X[0:48, :], in_=xk[0:48, :]).then_inc(in1, 16)
            sync.wait_ge(cpv, 1)
            sync.wait_ge(cps, 1)
            sync.dma_start(out=out[0:32, :], in_=S[0:32, :]).then_inc(outs, 16)

        @block.scalar
        def _(scalar):
            scalar.dma_start(out=X[48:80, :], in_=xk[48:80, :]).then_inc(in2, 16)
            # Dummy copy pulls ACT_TABLE_LOAD off the post-matmul critical path.
            scalar.copy(scratch_a[0:1, :], scratch_b[0:1, :])
            scalar.wait_ge(mm_sem, 1)
            scalar.copy(dst_view(1), src_view(1)).then_inc(cps, 1)
            scalar.wait_ge(cps, 1)
            scalar.wait_ge(cpv, 1)
            scalar.dma_start(out=out[32:48, :], in_=S[32:48, :]).then_inc(outs, 16)

        @block.vector
        def _(vector):
            vector.wait_ge(mm_sem, 1)
            vector.tensor_copy(dst_view(0), src_view(0)).then_inc(cpv, 1)

        @block.tensor
        def _(tensor):
            tensor.wait_ge(in1, 16)
            tensor.wait_ge(in2, 16)
            for kh in range(3):
                mm = tensor.matmul(
                    PS[0:M, 0:512],
                    lhsT=X[:, PCOLS + M * kh : PCOLS + M * (kh + 1)],
                    rhs=X[:, BW * kh : BW * kh + 512],
                    start=(kh == 0),
                    stop=(kh == 2),
                )
            mm.then_inc(mm_sem, 1)

    return nc


def make_in_maps(x, up_w, up_b, out_b):
    """Per-core [K, XCOLS] bf16: kw-im2col patch bands + block-diag weights."""
    import ml_dtypes

    x = np.asarray(x, dtype=np.float32)
    up_w = np.asarray(up_w, dtype=np.float32)
    up_b = np.asarray(up_b, dtype=np.float32)
    out_b = np.asarray(out_b, dtype=np.float32)

    # weights: wk[kh][10j + 3ci + kw, 48dr + 24dc + 3j + co]
    wk = np.zeros((3, K, M), dtype=np.float32)
    for j in range(NB):
        for co in range(C):
            for dr in range(2):
                for dc in range(2):
                    o = co * 4 + dr * 2 + dc
                    col = 64 * dr + 24 * dc + 3 * j + co
                    for ci in range(C):
                        for kw in range(3):
                            wk[:, 10 * j + 3 * ci + kw, col] = up_w[o, ci, :, kw]
                    wk[1, 10 * j + 9, col] = up_b[o] + out_b[co]
    wflat = wk.transpose(1, 0, 2).reshape(K, WCOLS)  # cols (kh, m)

    xpad = np.zeros((B, C, H + 2, W + 2), dtype=np.float32)
    xpad[:, :, 1 : H + 1, 1 : W + 1] = x

    in_maps = []
    for i in range(N_CORES):
        xc = np.empty((K, XCOLS), dtype=np.float32)
        xc[:, PCOLS:] = wflat
        for j in range(NB):
            b, rh, wh = j // 4, (j % 4) // 2, j % 2
            r0 = 16 * i + 8 * rh
            for ci in range(C):
                for kw in range(3):
                    xc[10 * j + 3 * ci + kw, :PCOLS] = xpad[
                        b, ci, r0 : r0 + 10, BW * wh + kw : BW * wh + kw + BW
                    ].reshape(PCOLS)
            xc[10 * j + 9, :PCOLS] = 1.0
        in_maps.append({"xk": xc.astype(ml_dtypes.bfloat16)})
    return in_maps


def kernel(x, up_w, up_b, in_w, in_b, adder_w, out_w, out_b):
    nc = build_graph()
    in_maps = make_in_maps(x, up_w, up_b, out_b)
    res = run_bass_kernel_spmd(nc, in_maps, core_ids=list(range(N_CORES)))
    slabs = []
    for i in range(N_CORES):
        a = np.asarray(res.results[i]["out"])  # [48 = (dc b rh wh co), cols]
        a = a.reshape(2, 2, 2, 2, 3, 4, 2, 2, 64)  # dc b rh wh co rrq dr whi wlo
        a = a.transpose(1, 4, 2, 5, 7, 6, 3, 8, 0)  # b co rh rrq whi dr wh wlo dc
        a = a.reshape(2, 3, 32, 256)
        slabs.append(a)
    return np.concatenate(slabs, axis=2).astype(np.float32)


# revision 8
# speedup vs baseline: 1.2329x; 1.0954x over previous
"""AdderVDSR kernel v2 for 8 TRN2 NeuronCores.

Mathematical collapse (see baseline): every AdderNet block outputs exactly 0
in fp32, so reference == pixel_shuffle(conv3(x, up_w, up_b), 2) + out_b.

v2 layout: block-diagonal band stacking.  Core i handles pre-shuffle rows
[16i, 16i+16) of both batches = 4096 pixels, split into 8 bands of 4 rows x
128 cols (band j = (batch, quad)).  Band j owns SBUF partitions [10j, 10j+10):
rows (ci, kw) = host-side im2col over input-channel and kw only (9 rows) plus
a ones row (bias).  The kh taps are free-dim COLUMN SHIFTS of one stored
[80, 768] patch tensor (6 rows x 128 per band incl. halo), so the whole conv
is 3 accumulating matmuls [80, 96] x [80, 512] into one PSUM bank -- 1536 PE
column-cycles total (baseline: 8192).  M order (dr, dc, band, color) makes
the 4 pixel-shuffle interleave copies contiguous-partition [24, 512] slices,
and the output SBUF/DRAM layout [24 = (b, quad, color), 2048] gives 24
contiguous 8KB DMA descriptors split across the two HWDGE queues (SP + ACT).
Host reshapes the flat [24, 2048] per-core result to [2, 3, 32, 256].
"""

import os

os.environ["CONCOURSE_SCRUB_NEFF_DEBUG_INFO"] = "1"

import numpy as np

import concourse.bass as bass
import concourse.mybir as mybir
from concourse.bass_utils import run_bass_kernel_spmd

N_CORES = 8
B, C, H, W = 2, 3, 128, 128
NB = 8                       # bands per core: (batch, row-half, col-half)
KPB = 10                     # partitions per band: 3 ci x 3 kw + ones
K = NB * KPB                 # 80 contraction partitions
M = 128                      # two 64-aligned dr groups of 48 used cols
BW = 64                      # band width (cols); band = 8 rows x 64 cols
PCOLS = 10 * BW              # 640 patch cols per partition (10 rows x 64)
WCOLS = 3 * M                # 384 weight cols (3 kh blocks of 128)
XCOLS = PCOLS + WCOLS        # 1024

_f32 = mybir.dt.float32
_bf16 = mybir.dt.bfloat16


def build_graph():
    nc = bass.Bass(disable_frame_to_traceback=True)
    xk = nc.declare_dram_parameter("xk", [K, XCOLS], _bf16, isOutput=False)
    out = nc.declare_dram_parameter("out", [48, 1024], _f32, isOutput=True)

    with (
        nc.sbuf_tensor([K, XCOLS], _bf16) as X,
        nc.sbuf_tensor([48, 1024], _f32) as S,
        nc.sbuf_tensor([1, 16], _f32) as scratch_a,
        nc.sbuf_tensor([1, 16], _f32) as scratch_b,
        nc.psum_tensor([M, 512], _f32) as PS,
        nc.semaphore("in1") as in1,
        nc.semaphore("in2") as in2,
        nc.semaphore("mm_sem") as mm_sem,
        nc.semaphore("cpd") as cpd,
        nc.semaphore("outs") as outs,
        nc.Block() as block,
    ):
        S3 = S.rearrange("p (rr x) -> p rr x", rr=4, x=256)
        PS3 = PS.rearrange("p (rr w) -> p rr w", rr=4, w=128)

        def dst_view(dr):
            # col = rr*256 + dr*128 + w (host interleaves w/dc and splits dc later)
            return S3[0:48, :, 128 * dr : 128 * dr + 128]

        def src_view(dr):
            return PS3[64 * dr : 64 * dr + 48, :, :]

        @block.sync
        def _(sync):
            sync.dma_start(out=X[0:48, :], in_=xk[0:48, :]).then_inc(in1, 16)
            sync.wait_ge(cpd, 2)
            sync.dma_start(out=out[0:32, :], in_=S[0:32, :]).then_inc(outs, 16)

        @block.scalar
        def _(scalar):
            scalar.dma_start(out=X[48:80, :], in_=xk[48:80, :]).then_inc(in2, 16)
            # Dummy copy pulls ACT_TABLE_LOAD off the post-matmul critical path.
            scalar.copy(scratch_a[0:1, :], scratch_b[0:1, :])
            scalar.wait_ge(mm_sem, 1)
            scalar.copy(dst_view(1), src_view(1)).then_inc(cpd, 1)
            scalar.wait_ge(cpd, 2)
            scalar.dma_start(out=out[32:48, :], in_=S[32:48, :]).then_inc(outs, 16)

        @block.vector
        def _(vector):
            vector.wait_ge(mm_sem, 1)
            vector.tensor_copy(dst_view(0), src_view(0)).then_inc(cpd, 1)

        @block.tensor
        def _(tensor):
            tensor.wait_ge(in1, 16)
            tensor.wait_ge(in2, 16)
            for kh in range(3):
                mm = tensor.matmul(
                    PS[0:M, 0:512],
                    lhsT=X[:, PCOLS + M * kh : PCOLS + M * (kh + 1)],
                    rhs=X[:, BW * kh : BW * kh + 512],
                    start=(kh == 0),
                    stop=(kh == 2),
                )
            mm.then_inc(mm_sem, 1)

    return nc


def make_in_maps(x, up_w, up_b, out_b):
    """Per-core [K, XCOLS] bf16: kw-im2col patch bands + block-diag weights."""
    import ml_dtypes

    x = np.asarray(x, dtype=np.float32)
    up_w = np.asarray(up_w, dtype=np.float32)
    up_b = np.asarray(up_b, dtype=np.float32)
    out_b = np.asarray(out_b, dtype=np.float32)

    # weights: wk[kh][10j + 3ci + kw, 48dr + 24dc + 3j + co]
    wk = np.zeros((3, K, M), dtype=np.float32)
    for j in range(NB):
        for co in range(C):
            for dr in range(2):
                for dc in range(2):
                    o = co * 4 + dr * 2 + dc
                    col = 64 * dr + 24 * dc + 3 * j + co
                    for ci in range(C):
                        for kw in range(3):
                            wk[:, 10 * j + 3 * ci + kw, col] = up_w[o, ci, :, kw]
                    wk[1, 10 * j + 9, col] = up_b[o] + out_b[co]
    wflat = wk.transpose(1, 0, 2).reshape(K, WCOLS)  # cols (kh, m)

    xpad = np.zeros((B, C, H + 2, W + 2), dtype=np.float32)
    xpad[:, :, 1 : H + 1, 1 : W + 1] = x

    in_maps = []
    for i in range(N_CORES):
        xc = np.empty((K, XCOLS), dtype=np.float32)
        xc[:, PCOLS:] = wflat
        for j in range(NB):
            b, rh, wh = j // 4, (j % 4) // 2, j % 2
            r0 = 16 * i + 8 * rh
            for ci in range(C):
                for kw in range(3):
                    xc[10 * j + 3 * ci + kw, :PCOLS] = xpad[
                        b, ci, r0 : r0 + 10, BW * wh + kw : BW * wh + kw + BW
                    ].reshape(PCOLS)
            xc[10 * j + 9, :PCOLS] = 1.0
        in_maps.append({"xk": xc.astype(ml_dtypes.bfloat16)})
    return in_maps


def kernel(x, up_w, up_b, in_w, in_b, adder_w, out_w, out_b):
    nc = build_graph()
    in_maps = make_in_maps(x, up_w, up_b, out_b)
    res = run_bass_kernel_spmd(nc, in_maps, core_ids=list(range(N_CORES)))
    slabs = []
    for i in range(N_CORES):
        a = np.asarray(res.results[i]["out"])  # [48 = (dc b rh wh co), cols]
        a = a.reshape(2, 2, 2, 2, 3, 4, 2, 2, 64)  # dc b rh wh co rrq dr whi wlo
        a = a.transpose(1, 4, 2, 5, 7, 6, 3, 8, 0)  # b co rh rrq whi dr wh wlo dc
        a = a.reshape(2, 3, 32, 256)
        slabs.append(a)
    return np.concatenate(slabs, axis=2).astype(np.float32)


# revision 9
# speedup vs baseline: 1.2480x; 1.0122x over previous
"""AdderVDSR kernel v2 for 8 TRN2 NeuronCores.

Mathematical collapse (see baseline): every AdderNet block outputs exactly 0
in fp32, so reference == pixel_shuffle(conv3(x, up_w, up_b), 2) + out_b.

v2 layout: block-diagonal band stacking.  Core i handles pre-shuffle rows
[16i, 16i+16) of both batches = 4096 pixels, split into 8 bands of 4 rows x
128 cols (band j = (batch, quad)).  Band j owns SBUF partitions [10j, 10j+10):
rows (ci, kw) = host-side im2col over input-channel and kw only (9 rows) plus
a ones row (bias).  The kh taps are free-dim COLUMN SHIFTS of one stored
[80, 768] patch tensor (6 rows x 128 per band incl. halo), so the whole conv
is 3 accumulating matmuls [80, 96] x [80, 512] into one PSUM bank -- 1536 PE
column-cycles total (baseline: 8192).  M order (dr, dc, band, color) makes
the 4 pixel-shuffle interleave copies contiguous-partition [24, 512] slices,
and the output SBUF/DRAM layout [24 = (b, quad, color), 2048] gives 24
contiguous 8KB DMA descriptors split across the two HWDGE queues (SP + ACT).
Host reshapes the flat [24, 2048] per-core result to [2, 3, 32, 256].
"""

import os

os.environ["CONCOURSE_SCRUB_NEFF_DEBUG_INFO"] = "1"

import numpy as np

import concourse.bass as bass
import concourse.mybir as mybir
from concourse.bass_utils import run_bass_kernel_spmd

N_CORES = 8
B, C, H, W = 2, 3, 128, 128
NB = 8                       # bands per core: (batch, row-half, col-half)
KPB = 10                     # partitions per band: 3 ci x 3 kw + ones
K = NB * KPB                 # 80 contraction partitions
M = 128                      # two 64-aligned dr groups of 48 used cols
BW = 64                      # band width (cols); band = 8 rows x 64 cols
PCOLS = 10 * BW              # 640 patch cols per partition (10 rows x 64)
WCOLS = 3 * M                # 384 weight cols (3 kh blocks of 128)
XCOLS = PCOLS + WCOLS        # 1024

_f32 = mybir.dt.float32
_bf16 = mybir.dt.bfloat16


def build_graph():
    nc = bass.Bass(disable_frame_to_traceback=True)
    xk = nc.declare_dram_parameter("xk", [K, XCOLS], _bf16, isOutput=False)
    out = nc.declare_dram_parameter("out", [48, 1024], _f32, isOutput=True)

    with (
        nc.sbuf_tensor([K, XCOLS], _bf16) as X,
        nc.sbuf_tensor([48, 1024], _f32) as S,
        nc.sbuf_tensor([1, 16], _f32) as scratch_a,
        nc.sbuf_tensor([1, 16], _f32) as scratch_b,
        nc.psum_tensor([M, 512], _f32) as PS,
        nc.semaphore("in1") as in1,
        nc.semaphore("in2") as in2,
        nc.semaphore("mm_sem") as mm_sem,
        nc.semaphore("cpd") as cpd,
        nc.semaphore("outs") as outs,
        nc.Block() as block,
    ):
        S3 = S.rearrange("p (rr x) -> p rr x", rr=4, x=256)
        PS3 = PS.rearrange("p (rr w) -> p rr w", rr=4, w=128)

        def dst_view(dr):
            # col = rr*256 + dr*128 + w (host interleaves w/dc and splits dc later)
            return S3[0:48, :, 128 * dr : 128 * dr + 128]

        def src_view(dr):
            return PS3[64 * dr : 64 * dr + 48, :, :]

        hoist = []

        @block.sync
        def _(sync):
            hoist.append(sync.dma_start(out=X[0:48, :], in_=xk[0:48, :]).then_inc(in1, 16))
            sync.wait_ge(cpd, 2)
            sync.dma_start(out=out[0:32, :], in_=S[0:32, :]).then_inc(outs, 16)

        @block.scalar
        def _(scalar):
            hoist.append(scalar.dma_start(out=X[48:80, :], in_=xk[48:80, :]).then_inc(in2, 16))
            # Dummy copy pulls ACT_TABLE_LOAD off the post-matmul critical path.
            scalar.copy(scratch_a[0:1, :], scratch_b[0:1, :])
            scalar.wait_ge(mm_sem, 1)
            scalar.copy(dst_view(1), src_view(1)).then_inc(cpd, 1)
            scalar.wait_ge(cpd, 2)
            scalar.dma_start(out=out[32:48, :], in_=S[32:48, :]).then_inc(outs, 16)

        @block.vector
        def _(vector):
            vector.wait_ge(mm_sem, 1)
            vector.tensor_copy(dst_view(0), src_view(0)).then_inc(cpd, 1)

        @block.tensor
        def _(tensor):
            tensor.wait_ge(in1, 16)
            tensor.wait_ge(in2, 16)
            for kh in range(3):
                mm = tensor.matmul(
                    PS[0:M, 0:512],
                    lhsT=X[:, PCOLS + M * kh : PCOLS + M * (kh + 1)],
                    rhs=X[:, BW * kh : BW * kh + 512],
                    start=(kh == 0),
                    stop=(kh == 2),
                )
            mm.then_inc(mm_sem, 1)

        # Hoist the input DMA issues above the framework's entry barrier:
        # they depend only on the init-time sem_clear fence, so re-home them
        # to just after gpsimd.preamble_end (same slot bacc uses), letting
        # SP/ACT issue while GpSimd runs the const memsets + barrier.
        f = nc.m.functions[0]
        insts = [h.ins if hasattr(h, "ins") else h for h in hoist]
        for blk in f.blocks:
            for inst in list(blk.instructions):
                if inst in insts:
                    blk.instructions.remove(inst)
        entry = f.blocks[0]
        idx = entry.instructions.index(nc.gpsimd.preamble_end) + 1
        for inst in reversed(insts):
            entry.instructions.insert(idx, inst)

    return nc


def make_in_maps(x, up_w, up_b, out_b):
    """Per-core [K, XCOLS] bf16: kw-im2col patch bands + block-diag weights."""
    import ml_dtypes

    x = np.asarray(x, dtype=np.float32)
    up_w = np.asarray(up_w, dtype=np.float32)
    up_b = np.asarray(up_b, dtype=np.float32)
    out_b = np.asarray(out_b, dtype=np.float32)

    # weights: wk[kh][10j + 3ci + kw, 48dr + 24dc + 3j + co]
    wk = np.zeros((3, K, M), dtype=np.float32)
    for j in range(NB):
        for co in range(C):
            for dr in range(2):
                for dc in range(2):
                    o = co * 4 + dr * 2 + dc
                    col = 64 * dr + 24 * dc + 3 * j + co
                    for ci in range(C):
                        for kw in range(3):
                            wk[:, 10 * j + 3 * ci + kw, col] = up_w[o, ci, :, kw]
                    wk[1, 10 * j + 9, col] = up_b[o] + out_b[co]
    wflat = wk.transpose(1, 0, 2).reshape(K, WCOLS)  # cols (kh, m)

    xpad = np.zeros((B, C, H + 2, W + 2), dtype=np.float32)
    xpad[:, :, 1 : H + 1, 1 : W + 1] = x

    in_maps = []
    for i in range(N_CORES):
        xc = np.empty((K, XCOLS), dtype=np.float32)
        xc[:, PCOLS:] = wflat
        for j in range(NB):
            b, rh, wh = j // 4, (j % 4) // 2, j % 2
            r0 = 16 * i + 8 * rh
            for ci in range(C):
                for kw in range(3):
                    xc[10 * j + 3 * ci + kw, :PCOLS] = xpad[
                        b, ci, r0 : r0 + 10, BW * wh + kw : BW * wh + kw + BW
                    ].reshape(PCOLS)
            xc[10 * j + 9, :PCOLS] = 1.0
        in_maps.append({"xk": xc.astype(ml_dtypes.bfloat16)})
    return in_maps


def kernel(x, up_w, up_b, in_w, in_b, adder_w, out_w, out_b):
    nc = build_graph()
    in_maps = make_in_maps(x, up_w, up_b, out_b)
    res = run_bass_kernel_spmd(nc, in_maps, core_ids=list(range(N_CORES)))
    slabs = []
    for i in range(N_CORES):
        a = np.asarray(res.results[i]["out"])  # [48 = (dc b rh wh co), cols]
        a = a.reshape(2, 2, 2, 2, 3, 4, 2, 2, 64)  # dc b rh wh co rrq dr whi wlo
        a = a.transpose(1, 4, 2, 5, 7, 6, 3, 8, 0)  # b co rh rrq whi dr wh wlo dc
        a = a.reshape(2, 3, 32, 256)
        slabs.append(a)
    return np.concatenate(slabs, axis=2).astype(np.float32)
